# revision 24
# baseline (speedup 1.0000x reference)
"""Multi-head self-attention with RoPE on 8 Trainium2 NeuronCores.

Sharding: core c = batch(c // 4) x head-group(c % 4) -> 4 heads per core.
Each core computes attention for its 4 heads and a partial O-projection
(full [S, D] output restricted to its 256 input features); the host sums
the 4 partials per batch.

Device layout tricks:
  * x is transposed on host -> xT [D, S]; all projection matmuls contract
    over partitions without any on-device transpose.
  * Wq/Wk rows are permuted on host so even rope dims (E) and odd rope
    dims (O) of the 4 heads land in two separate 128-row projection
    outputs. RoPE then becomes lane-aligned elementwise DVE ops, and the
    Q.K contraction (invariant to the shared permutation) is done as two
    accumulating K=32 matmuls per head at distinct PE row-groups.
  * Scores are computed transposed (S_T[k, q]) so P_T feeds the PV matmul
    as the moving operand; a ones-column appended to V accumulates the
    softmax denominator in the same matmul. Softmax skips the max
    subtraction (scores are bounded ~|5|), exactly like exp-sum-divide.
  * All matmul operands are bf16 (fp32 PSUM accumulation): f32r moving
    operands stream at half rate and block fast-weight-load.
  * Projection (s-block sb) and attention (q-block sb) are interleaved in
    one loop so the PE always has independent matmul work -> HAM stays at
    K=8/8 (2.4 GHz) instead of oscillating cold.
  * exp runs on ACT for even heads and as a one-instruction Schraudolph
    bf16-bit-trick tensor_scalar on DVE for odd heads, halving exp wall
    time. Softmax normalizes with the same approximated values, so the
    denominator stays consistent.
"""

import os
import sys

sys.path.insert(0, "/opt/trn_rl_repo")

from contextlib import ExitStack

import ml_dtypes
import numpy as np

import concourse.bass as bass
import concourse.tile as tile
from concourse import bacc, mybir
from concourse.bass_utils import run_bass_kernel_spmd

B = 2
S = 4096
D = 1024
NH = 16
DK = 64
HPC = 4  # heads per core
N_CORES = 8
THETA = 10000.0
SBLK = 512  # s-block / q-block width
NSB = S // SBLK
KC = 128  # k chunk
F32 = mybir.dt.float32
BF16 = mybir.dt.bfloat16
I16 = mybir.dt.int16
NPBF = ml_dtypes.bfloat16
EXP = mybir.ActivationFunctionType.Exp
MULT = mybir.AluOpType.mult
ADD = mybir.AluOpType.add
# Schraudolph exp as bf16 bits: int16(x*0.125*128*log2e + 128*(127-C))
SCH_A = 128.0 * 1.4426950408889634 * 0.125
SCH_B = 128.0 * (127.0 - 0.04367)

_PROGRAM = None


def _emit(nc, loop_n=1):
    xT = nc.dram_tensor("xT", [D, S], BF16, kind="ExternalInput").ap()
    wqeT = nc.dram_tensor("wqeT", [D, 128], BF16, kind="ExternalInput").ap()
    wqoT = nc.dram_tensor("wqoT", [D, 128], BF16, kind="ExternalInput").ap()
    wkeT = nc.dram_tensor("wkeT", [D, 128], BF16, kind="ExternalInput").ap()
    wkoT = nc.dram_tensor("wkoT", [D, 128], BF16, kind="ExternalInput").ap()
    wvT = nc.dram_tensor("wvT", [D, 256], BF16, kind="ExternalInput").ap()
    woT = nc.dram_tensor("woT", [256, D], BF16, kind="ExternalInput").ap()
    cos4 = nc.dram_tensor("cos4", [128, S], F32, kind="ExternalInput").ap()
    sin4 = nc.dram_tensor("sin4", [128, S], F32, kind="ExternalInput").ap()
    masks = nc.dram_tensor("masks", [128, 128], BF16, kind="ExternalInput").ap()
    outp = nc.dram_tensor("out", [S, D], BF16, kind="ExternalOutput").ap()

    with tile.TileContext(nc) as tc, ExitStack() as ctx:
        wpool = ctx.enter_context(tc.tile_pool(name="w", bufs=1))
        xpool = ctx.enter_context(tc.tile_pool(name="x", bufs=24))
        qkpool = ctx.enter_context(tc.tile_pool(name="qk", bufs=1))
        vpool = ctx.enter_context(tc.tile_pool(name="v", bufs=1))
        trig = ctx.enter_context(tc.tile_pool(name="trig", bufs=4))
        tmp = ctx.enter_context(tc.tile_pool(name="tmp", bufs=2))
        ppool = ctx.enter_context(tc.tile_pool(name="p", bufs=6))
        apool = ctx.enter_context(tc.tile_pool(name="a", bufs=4))
        rpool = ctx.enter_context(tc.tile_pool(name="r", bufs=4))
        rbpool = ctx.enter_context(tc.tile_pool(name="rb", bufs=4))
        opool = ctx.enter_context(tc.tile_pool(name="ot", bufs=3))

        # ---- persistent SBUF tensors ----
        w_qe = wpool.tile([128, 8 * 128], BF16, tag="wqe")
        w_qo = wpool.tile([128, 8 * 128], BF16, tag="wqo")
        w_ke = wpool.tile([128, 8 * 128], BF16, tag="wke")
        w_ko = wpool.tile([128, 8 * 128], BF16, tag="wko")
        w_v = wpool.tile([128, 8 * 256], BF16, tag="wv")
        w_o0 = wpool.tile([128, D], BF16, tag="wo0")
        w_o1 = wpool.tile([128, D], BF16, tag="wo1")
        mask_t = wpool.tile([128, 128], BF16, tag="mask")

        for dst, src in (
            (w_qe, wqeT),
            (w_qo, wqoT),
            (w_ke, wkeT),
            (w_ko, wkoT),
            (w_v, wvT),
        ):
            nc.sync.dma_start(
                dst[:].rearrange("p (i f) -> p i f", i=8),
                src.rearrange("(i p) f -> p i f", p=128),
            )
        nc.sync.dma_start(w_o0[:], woT[0:128, :])
        nc.sync.dma_start(w_o1[:], woT[128:256, :])
        nc.sync.dma_start(mask_t[:], masks)

        qtE = qkpool.tile([128, S], BF16, tag="qtE")
        qtO = qkpool.tile([128, S], BF16, tag="qtO")
        ktE = qkpool.tile([128, S], BF16, tag="ktE")
        ktO = qkpool.tile([128, S], BF16, tag="ktO")
        # V with a ones column appended per head: [.. V_h (64) | 1 ..] x4
        vbig = vpool.tile([128, 32 * 260], BF16, tag="vbig")
        nc.vector.memset(vbig[:], 1.0)

        # ---- phase B: QKV projections + rope ----
        loop_ctx = tc.For_i(0, loop_n, 1) if loop_n > 1 else None
        if loop_ctx is not None:
            loop_ctx.__enter__()
        with (
            tc.tile_pool(name="projps", bufs=2, space="PSUM") as projps,
            tc.tile_pool(name="sps", bufs=2, space="PSUM") as sps,
            tc.tile_pool(name="accps", bufs=2, space="PSUM") as accps,
        ):
            for sb in range(NSB):
                scol = slice(sb * SBLK, (sb + 1) * SBLK)
                xt = []
                for i in range(8):
                    t = xpool.tile([128, SBLK], BF16, tag="xt")
                    nc.sync.dma_start(t[:], xT[i * 128 : (i + 1) * 128, scol])
                    xt.append(t)
                cos_t = trig.tile([128, SBLK], F32, tag="cos")
                sin_t = trig.tile([128, SBLK], F32, tag="sin")
                nc.sync.dma_start(cos_t[:], cos4[:, scol])
                nc.sync.dma_start(sin_t[:], sin4[:, scol])

                ps = {}
                for name, w in (("qe", w_qe), ("qo", w_qo), ("ke", w_ke), ("ko", w_ko)):
                    p = projps.tile([128, SBLK], F32, tag="proj")
                    for i in range(8):
                        nc.tensor.matmul(
                            p[:],
                            w[:, i * 128 : (i + 1) * 128],
                            xt[i][:],
                            start=(i == 0),
                            stop=(i == 7),
                        )
                    ps[name] = p

                # rope: E' = E cos - O sin ; O' = E sin + O cos
                for pe, po, dE, dO in (
                    (ps["qe"], ps["qo"], qtE, qtO),
                    (ps["ke"], ps["ko"], ktE, ktO),
                ):
                    t1 = tmp.tile([128, SBLK], F32, tag="t1")
                    t2 = tmp.tile([128, SBLK], F32, tag="t2")
                    nc.vector.tensor_mul(t1[:], pe[:], cos_t[:])
                    nc.vector.tensor_mul(t2[:], po[:], sin_t[:])
                    nc.vector.tensor_sub(dE[:, scol], t1[:], t2[:])
                    t3 = tmp.tile([128, SBLK], F32, tag="t1")
                    t4 = tmp.tile([128, SBLK], F32, tag="t2")
                    nc.vector.tensor_mul(t3[:], pe[:], sin_t[:])
                    nc.vector.tensor_mul(t4[:], po[:], cos_t[:])
                    nc.vector.tensor_add(dO[:, scol], t3[:], t4[:])

                # V for the 4 k-chunks of this s-block
                for ss in range(4):
                    vp = projps.tile([128, SBLK], F32, tag="proj")
                    for i in range(8):
                        nc.tensor.matmul(
                            vp[:, 0:256],
                            xt[i][:, ss * 128 : (ss + 1) * 128],
                            w_v[:, i * 256 : (i + 1) * 256],
                            start=(i == 0),
                            stop=(i == 7),
                        )
                    kc = sb * 4 + ss
                    nc.scalar.copy(
                        vbig[:, kc * 260 : (kc + 1) * 260].rearrange(
                            "p (h f) -> p h f", h=4
                        )[:, :, 0:64],
                        vp[:, 0:256].rearrange("p (h f) -> p h f", h=4),
                    )

                # ---- attention for q-block qb = sb ----
                qb = sb
                a_tiles = [
                    apool.tile([128, SBLK], BF16, tag="a", name=f"a{qb}_{i}")
                    for i in range(2)
                ]
                nk = 4 * qb + 4
                for hp in range(2):
                    heads = (2 * hp, 2 * hp + 1)
                    accs = [
                        accps.tile([65, SBLK], F32, tag="acc", name=f"acc{qb}_{h}")
                        for h in heads
                    ]
                    for cc in range(nk // 2):
                        chunks = (2 * cc, 2 * cc + 1)
                        # d = masked-out leading q-columns of each chunk
                        ds = [max(0, (c - 4 * qb)) * KC for c in chunks]
                        sp_pair = [
                            sps.tile(
                                [128, 2 * SBLK], F32, tag="s", name=f"s{qb}_{cc}_{h}"
                            )
                            for h in heads
                        ]
                        # head-inner order: consecutive matmuls hit different
                        # PE row-groups so they execute concurrently
                        for kt, qt, st0, st1 in (
                            (ktE, qtE, True, False),
                            (ktO, qtO, False, True),
                        ):
                            for ci, c in enumerate(chunks):
                                d = ds[ci]
                                for hi, h in enumerate(heads):
                                    hr = slice(h * 32, (h + 1) * 32)
                                    nc.tensor.matmul(
                                        sp_pair[hi][:, ci * SBLK + d : (ci + 1) * SBLK],
                                        kt[hr, c * KC : (c + 1) * KC],
                                        qt[hr, qb * SBLK + d : (qb + 1) * SBLK],
                                        start=st0,
                                        stop=st1,
                                        tile_position=(h * 32, 0),
                                    )
                        pts = []
                        for hi, h in enumerate(heads):
                            pt = ppool.tile([128, 2 * SBLK], BF16, tag="pt")
                            if hi == 0:
                                nc.scalar.activation(
                                    pt[:, ds[0] :],
                                    sp_pair[hi][:, ds[0] :],
                                    EXP,
                                    scale=0.125,
                                )
                            else:
                                nc.vector.tensor_scalar(
                                    pt[:, ds[0] :].bitcast(I16),
                                    sp_pair[hi][:, ds[0] :],
                                    SCH_A,
                                    SCH_B,
                                    op0=MULT,
                                    op1=ADD,
                                )
                            for ci, c in enumerate(chunks):
                                d = ds[ci]
                                if c >= 4 * qb:
                                    # diagonal 128x128 triangle of this chunk
                                    nc.vector.tensor_mul(
                                        pt[:, ci * SBLK + d : ci * SBLK + d + KC],
                                        pt[:, ci * SBLK + d : ci * SBLK + d + KC],
                                        mask_t[:],
                                    )
                            pts.append(pt)
                        for hi, h in enumerate(heads):
                            pt = pts[hi]
                            for ci, c in enumerate(chunks):
                                d = ds[ci]
                                nc.tensor.matmul(
                                    accs[hi][0:65, d:SBLK],
                                    vbig[:, c * 260 + h * 65 : c * 260 + (h + 1) * 65],
                                    pt[:, ci * SBLK + d : (ci + 1) * SBLK],
                                    start=(c == 0),
                                    stop=(c == nk - 1),
                                )
                    for hi, h in enumerate(heads):
                        den = rpool.tile([1, SBLK], F32, tag="den")
                        nc.scalar.copy(den[:], accs[hi][64:65, :])
                        recip = rpool.tile([1, SBLK], F32, tag="recip")
                        nc.vector.reciprocal_approx_fast(recip[:], den[:])
                        rb = rbpool.tile([64, SBLK], F32, tag="rb")
                        nc.gpsimd.partition_broadcast(rb[:], recip[0:1, :])
                        nc.vector.tensor_mul(
                            a_tiles[h // 2][(h % 2) * 64 : (h % 2 + 1) * 64, :],
                            accs[hi][0:64, :],
                            rb[:],
                        )
                # O projection for this q-block (borrows sps psum slots)
                for ss in range(4):
                    r0 = qb * SBLK + ss * 128
                    op = sps.tile([128, 2 * SBLK], F32, tag="s", name=f"o{qb}_{ss}")
                    for ob in range(2):
                        nc.tensor.matmul(
                            op[:, ob * 512 : (ob + 1) * 512],
                            a_tiles[0][:, ss * 128 : (ss + 1) * 128],
                            w_o0[:, ob * 512 : (ob + 1) * 512],
                            start=True,
                            stop=False,
                        )
                        nc.tensor.matmul(
                            op[:, ob * 512 : (ob + 1) * 512],
                            a_tiles[1][:, ss * 128 : (ss + 1) * 128],
                            w_o1[:, ob * 512 : (ob + 1) * 512],
                            start=False,
                            stop=True,
                        )
                    osb = opool.tile([128, D], BF16, tag="osb", name=f"osb{qb}_{ss}")
                    nc.scalar.copy(osb[:], op[:])
                    nc.sync.dma_start(outp[r0 : r0 + 128, :], osb[:])
        if loop_ctx is not None:
            loop_ctx.__exit__(None, None, None)


def _build(loop_n=1):
    global _PROGRAM
    if loop_n != 1:
        nc = bacc.Bacc(
            "TRN2", target_bir_lowering=False, debug=False, num_devices=N_CORES
        )
        _emit(nc, loop_n)
        nc.compile()
        return nc
    if _PROGRAM is None:
        nc = bacc.Bacc(
            "TRN2", target_bir_lowering=False, debug=False, num_devices=N_CORES
        )
        _emit(nc)
        nc.compile()
        _PROGRAM = nc
    return _PROGRAM


def _rope_caches():
    j = np.arange(0, DK, 2, dtype=np.float32) / np.float32(DK)
    freqs = (1.0 / THETA**j).astype(np.float32)  # [32]
    t = np.arange(S, dtype=np.float32)
    ang = np.outer(t, freqs).astype(np.float32)  # [S, 32]
    return np.cos(ang), np.sin(ang)


def _make_masks():
    kk = np.arange(128)[:, None]
    jj = np.arange(128)[None, :]
    return (kk <= jj).astype(NPBF)


def _make_in_maps(x, token_positions, Wq, Wk, Wv, Wo):
    x = np.asarray(x, dtype=np.float32)
    token_positions = np.asarray(token_positions)
    Wq, Wk, Wv, Wo = (np.asarray(w, dtype=np.float32) for w in (Wq, Wk, Wv, Wo))

    cos_c, sin_c = _rope_caches()
    masks = _make_masks()

    in_maps = []
    for c in range(N_CORES):
        b, g = divmod(c, 4)
        heads = [g * HPC + hh for hh in range(HPC)]
        rows_e = np.concatenate([h * DK + np.arange(0, DK, 2) for h in heads])
        rows_o = rows_e + 1
        rows_v = np.concatenate([h * DK + np.arange(DK) for h in heads])

        pos = np.asarray(token_positions[b], dtype=np.int64)
        cosb = np.ascontiguousarray(cos_c[pos].T)  # [32, S]
        sinb = np.ascontiguousarray(sin_c[pos].T)

        in_maps.append(
            {
                "xT": np.ascontiguousarray(x[b].T).astype(NPBF),
                "wqeT": np.ascontiguousarray(Wq[rows_e].T).astype(NPBF),
                "wqoT": np.ascontiguousarray(Wq[rows_o].T).astype(NPBF),
                "wkeT": np.ascontiguousarray(Wk[rows_e].T).astype(NPBF),
                "wkoT": np.ascontiguousarray(Wk[rows_o].T).astype(NPBF),
                "wvT": np.ascontiguousarray(Wv[rows_v].T).astype(NPBF),
                "woT": np.ascontiguousarray(Wo[:, rows_v].T).astype(NPBF),
                "cos4": np.ascontiguousarray(np.tile(cosb, (4, 1))),
                "sin4": np.ascontiguousarray(np.tile(sinb, (4, 1))),
                "masks": masks,
            }
        )
    return in_maps


def kernel(x, token_positions, Wq, Wk, Wv, Wo):
    nc = _build()
    in_maps = _make_in_maps(x, token_positions, Wq, Wk, Wv, Wo)
    res = run_bass_kernel_spmd(nc, in_maps, list(range(N_CORES)))
    out = np.zeros((B, S, D), dtype=np.float32)
    for c in range(N_CORES):
        out[c // 4] += np.asarray(res.results[c]["out"], dtype=np.float32)
    return out


# revision 26
# speedup vs baseline: 1.0859x; 1.0859x over previous
"""Multi-head self-attention with RoPE on 8 Trainium2 NeuronCores.

Sharding: core c = batch(c // 4) x head-group(c % 4) -> 4 heads per core.
Each core computes attention for its 4 heads and a partial O-projection
(full [S, D] output restricted to its 256 input features); the host sums
the 4 partials per batch.

Device layout tricks:
  * x is transposed on host -> xT [D, S]; all projection matmuls contract
    over partitions without any on-device transpose.
  * Wq/Wk rows are permuted on host so even rope dims (E) and odd rope
    dims (O) of the 4 heads land in two separate 128-row projection
    outputs. RoPE then becomes lane-aligned elementwise DVE ops, and the
    Q.K contraction (invariant to the shared permutation) is done as two
    accumulating K=32 matmuls per head at distinct PE row-groups.
  * Scores are computed transposed (S_T[k, q]) so P_T feeds the PV matmul
    as the moving operand; a ones-column appended to V accumulates the
    softmax denominator in the same matmul. Softmax skips the max
    subtraction (scores are bounded ~|5|), exactly like exp-sum-divide.
  * All matmul operands are bf16 (fp32 PSUM accumulation): f32r moving
    operands stream at half rate and block fast-weight-load.
  * Projection (s-block sb) and attention (q-block sb) are interleaved in
    one loop so the PE always has independent matmul work -> HAM stays at
    K=8/8 (2.4 GHz) instead of oscillating cold.
  * exp runs on ACT for even heads and as a one-instruction Schraudolph
    bf16-bit-trick tensor_scalar on DVE for odd heads, halving exp wall
    time. Softmax normalizes with the same approximated values, so the
    denominator stays consistent.
"""

import os
import sys

sys.path.insert(0, "/opt/trn_rl_repo")

from contextlib import ExitStack

import ml_dtypes
import numpy as np

import concourse.bass as bass
import concourse.tile as tile
from concourse import bacc, mybir
from concourse.bass_utils import run_bass_kernel_spmd

B = 2
S = 4096
D = 1024
NH = 16
DK = 64
HPC = 4  # heads per core
N_CORES = 8
THETA = 10000.0
SBLK = 512  # s-block / q-block width
NSB = S // SBLK
KC = 128  # k chunk
F32 = mybir.dt.float32
BF16 = mybir.dt.bfloat16
I16 = mybir.dt.int16
NPBF = ml_dtypes.bfloat16
EXP = mybir.ActivationFunctionType.Exp
MULT = mybir.AluOpType.mult
ADD = mybir.AluOpType.add
# Schraudolph exp as bf16 bits: int16(x*0.125*128*log2e + 128*(127-C))
SCH_A = 128.0 * 1.4426950408889634 * 0.125
SCH_B = 128.0 * (127.0 - 0.04367)

_PROGRAM = None


def _emit(nc, loop_n=1):
    xT = nc.dram_tensor("xT", [D, S], BF16, kind="ExternalInput").ap()
    wqeT = nc.dram_tensor("wqeT", [D, 128], BF16, kind="ExternalInput").ap()
    wqoT = nc.dram_tensor("wqoT", [D, 128], BF16, kind="ExternalInput").ap()
    wkeT = nc.dram_tensor("wkeT", [D, 128], BF16, kind="ExternalInput").ap()
    wkoT = nc.dram_tensor("wkoT", [D, 128], BF16, kind="ExternalInput").ap()
    wvT = nc.dram_tensor("wvT", [D, 256], BF16, kind="ExternalInput").ap()
    woT = nc.dram_tensor("woT", [256, D], BF16, kind="ExternalInput").ap()
    cos4 = nc.dram_tensor("cos4", [128, S], F32, kind="ExternalInput").ap()
    sin4 = nc.dram_tensor("sin4", [128, S], F32, kind="ExternalInput").ap()
    masks = nc.dram_tensor("masks", [128, 128], BF16, kind="ExternalInput").ap()
    outp = nc.dram_tensor("out", [S, D], BF16, kind="ExternalOutput").ap()

    with tile.TileContext(nc) as tc, ExitStack() as ctx:
        wpool = ctx.enter_context(tc.tile_pool(name="w", bufs=1))
        xpool = ctx.enter_context(tc.tile_pool(name="x", bufs=24))
        qkpool = ctx.enter_context(tc.tile_pool(name="qk", bufs=1))
        vpool = ctx.enter_context(tc.tile_pool(name="v", bufs=1))
        trig = ctx.enter_context(tc.tile_pool(name="trig", bufs=4))
        tmp = ctx.enter_context(tc.tile_pool(name="tmp", bufs=4))
        ppool = ctx.enter_context(tc.tile_pool(name="p", bufs=4))
        apool = ctx.enter_context(tc.tile_pool(name="a", bufs=4))
        rpool = ctx.enter_context(tc.tile_pool(name="r", bufs=4))
        rbpool = ctx.enter_context(tc.tile_pool(name="rb", bufs=2))
        opool = ctx.enter_context(tc.tile_pool(name="ot", bufs=3))

        # ---- persistent SBUF tensors ----
        w_qe = wpool.tile([128, 8 * 128], BF16, tag="wqe")
        w_qo = wpool.tile([128, 8 * 128], BF16, tag="wqo")
        w_ke = wpool.tile([128, 8 * 128], BF16, tag="wke")
        w_ko = wpool.tile([128, 8 * 128], BF16, tag="wko")
        w_v = wpool.tile([128, 8 * 256], BF16, tag="wv")
        w_o0 = wpool.tile([128, D], BF16, tag="wo0")
        w_o1 = wpool.tile([128, D], BF16, tag="wo1")
        mask_t = wpool.tile([128, 128], BF16, tag="mask")

        for dst, src in (
            (w_qe, wqeT),
            (w_qo, wqoT),
            (w_ke, wkeT),
            (w_ko, wkoT),
            (w_v, wvT),
        ):
            nc.sync.dma_start(
                dst[:].rearrange("p (i f) -> p i f", i=8),
                src.rearrange("(i p) f -> p i f", p=128),
            )
        nc.sync.dma_start(w_o0[:], woT[0:128, :])
        nc.sync.dma_start(w_o1[:], woT[128:256, :])
        nc.sync.dma_start(mask_t[:], masks)

        qtE = qkpool.tile([128, S], BF16, tag="qtE")
        qtO = qkpool.tile([128, S], BF16, tag="qtO")
        ktE = qkpool.tile([128, S], BF16, tag="ktE")
        ktO = qkpool.tile([128, S], BF16, tag="ktO")
        # V with a ones column appended per head: [.. V_h (64) | 1 ..] x4
        vbig = vpool.tile([128, 32 * 260], BF16, tag="vbig")
        nc.vector.memset(vbig[:], 1.0)

        # ---- phase B: QKV projections + rope ----
        loop_ctx = tc.For_i(0, loop_n, 1) if loop_n > 1 else None
        if loop_ctx is not None:
            loop_ctx.__enter__()
        with (
            tc.tile_pool(name="projps", bufs=2, space="PSUM") as projps,
            tc.tile_pool(name="sps", bufs=2, space="PSUM") as sps,
            tc.tile_pool(name="accps", bufs=2, space="PSUM") as accps,
        ):
            for sb in range(NSB):
                scol = slice(sb * SBLK, (sb + 1) * SBLK)
                xt = []
                for i in range(8):
                    t = xpool.tile([128, SBLK], BF16, tag="xt")
                    nc.sync.dma_start(t[:], xT[i * 128 : (i + 1) * 128, scol])
                    xt.append(t)
                cos_t = trig.tile([128, SBLK], F32, tag="cos")
                sin_t = trig.tile([128, SBLK], F32, tag="sin")
                nc.sync.dma_start(cos_t[:], cos4[:, scol])
                nc.sync.dma_start(sin_t[:], sin4[:, scol])

                ps = {}
                for name, w in (("qe", w_qe), ("qo", w_qo), ("ke", w_ke), ("ko", w_ko)):
                    p = projps.tile([128, SBLK], F32, tag="proj")
                    for i in range(8):
                        nc.tensor.matmul(
                            p[:],
                            w[:, i * 128 : (i + 1) * 128],
                            xt[i][:],
                            start=(i == 0),
                            stop=(i == 7),
                        )
                    ps[name] = p

                # rope: E' = E cos - O sin ; O' = E sin + O cos
                for pe, po, dE, dO in (
                    (ps["qe"], ps["qo"], qtE, qtO),
                    (ps["ke"], ps["ko"], ktE, ktO),
                ):
                    t1 = tmp.tile([128, SBLK], F32, tag="t1")
                    t2 = tmp.tile([128, SBLK], F32, tag="t2")
                    nc.vector.tensor_mul(t1[:], pe[:], cos_t[:])
                    nc.vector.tensor_mul(t2[:], po[:], sin_t[:])
                    nc.vector.tensor_sub(dE[:, scol], t1[:], t2[:])
                    t3 = tmp.tile([128, SBLK], F32, tag="t1")
                    t4 = tmp.tile([128, SBLK], F32, tag="t2")
                    nc.vector.tensor_mul(t3[:], pe[:], sin_t[:])
                    nc.vector.tensor_mul(t4[:], po[:], cos_t[:])
                    nc.vector.tensor_add(dO[:, scol], t3[:], t4[:])

                # V for the 4 k-chunks of this s-block
                for ss in range(4):
                    vp = projps.tile([128, SBLK], F32, tag="proj")
                    for i in range(8):
                        nc.tensor.matmul(
                            vp[:, 0:256],
                            xt[i][:, ss * 128 : (ss + 1) * 128],
                            w_v[:, i * 256 : (i + 1) * 256],
                            start=(i == 0),
                            stop=(i == 7),
                        )
                    kc = sb * 4 + ss
                    nc.vector.tensor_copy(
                        vbig[:, kc * 260 : (kc + 1) * 260].rearrange(
                            "p (h f) -> p h f", h=4
                        )[:, :, 0:64],
                        vp[:, 0:256].rearrange("p (h f) -> p h f", h=4),
                    )

                # ---- attention for q-block qb = sb ----
                qb = sb
                a_tiles = [
                    apool.tile([128, SBLK], BF16, tag="a", name=f"a{qb}_{i}")
                    for i in range(2)
                ]
                nk = 4 * qb + 4
                for hp in range(2):
                    heads = (2 * hp, 2 * hp + 1)
                    accs = [
                        accps.tile([65, SBLK], F32, tag="acc", name=f"acc{qb}_{h}")
                        for h in heads
                    ]
                    for cc in range(nk // 2):
                        chunks = (2 * cc, 2 * cc + 1)
                        # d = masked-out leading q-columns of each chunk
                        ds = [max(0, (c - 4 * qb)) * KC for c in chunks]
                        sp_pair = [
                            sps.tile(
                                [128, 2 * SBLK], F32, tag="s", name=f"s{qb}_{cc}_{h}"
                            )
                            for h in heads
                        ]
                        # head-inner order: consecutive matmuls hit different
                        # PE row-groups so they execute concurrently
                        for kt, qt, st0, st1 in (
                            (ktE, qtE, True, False),
                            (ktO, qtO, False, True),
                        ):
                            for ci, c in enumerate(chunks):
                                d = ds[ci]
                                for hi, h in enumerate(heads):
                                    hr = slice(h * 32, (h + 1) * 32)
                                    nc.tensor.matmul(
                                        sp_pair[hi][:, ci * SBLK + d : (ci + 1) * SBLK],
                                        kt[hr, c * KC : (c + 1) * KC],
                                        qt[hr, qb * SBLK + d : (qb + 1) * SBLK],
                                        start=st0,
                                        stop=st1,
                                        tile_position=(h * 32, 0),
                                    )
                        pts = []
                        for hi, h in enumerate(heads):
                            pt = ppool.tile([128, 2 * SBLK], BF16, tag="pt")
                            if hi == 0 or cc % 4 == 3:
                                nc.scalar.activation(
                                    pt[:, ds[0] :],
                                    sp_pair[hi][:, ds[0] :],
                                    EXP,
                                    scale=0.125,
                                )
                            else:
                                nc.vector.tensor_scalar(
                                    pt[:, ds[0] :].bitcast(I16),
                                    sp_pair[hi][:, ds[0] :],
                                    SCH_A,
                                    SCH_B,
                                    op0=MULT,
                                    op1=ADD,
                                )
                            for ci, c in enumerate(chunks):
                                d = ds[ci]
                                if c >= 4 * qb:
                                    # diagonal 128x128 triangle of this chunk
                                    nc.vector.tensor_mul(
                                        pt[:, ci * SBLK + d : ci * SBLK + d + KC],
                                        pt[:, ci * SBLK + d : ci * SBLK + d + KC],
                                        mask_t[:],
                                    )
                            pts.append(pt)
                        for hi, h in enumerate(heads):
                            pt = pts[hi]
                            for ci, c in enumerate(chunks):
                                d = ds[ci]
                                nc.tensor.matmul(
                                    accs[hi][0:65, d:SBLK],
                                    vbig[:, c * 260 + h * 65 : c * 260 + (h + 1) * 65],
                                    pt[:, ci * SBLK + d : (ci + 1) * SBLK],
                                    start=(c == 0),
                                    stop=(c == nk - 1),
                                )
                    for hi, h in enumerate(heads):
                        den = rpool.tile([1, SBLK], F32, tag="den")
                        nc.vector.tensor_copy(den[:], accs[hi][64:65, :])
                        recip = rpool.tile([1, SBLK], F32, tag="recip")
                        nc.vector.reciprocal_approx_fast(recip[:], den[:])
                        rb = rbpool.tile([64, SBLK], F32, tag="rb")
                        nc.gpsimd.partition_broadcast(rb[:], recip[0:1, :])
                        nc.vector.tensor_mul(
                            a_tiles[h // 2][(h % 2) * 64 : (h % 2 + 1) * 64, :],
                            accs[hi][0:64, :],
                            rb[:],
                        )
                # O projection for this q-block (borrows sps psum slots)
                for ss in range(4):
                    r0 = qb * SBLK + ss * 128
                    op = sps.tile([128, 2 * SBLK], F32, tag="s", name=f"o{qb}_{ss}")
                    for ob in range(2):
                        nc.tensor.matmul(
                            op[:, ob * 512 : (ob + 1) * 512],
                            a_tiles[0][:, ss * 128 : (ss + 1) * 128],
                            w_o0[:, ob * 512 : (ob + 1) * 512],
                            start=True,
                            stop=False,
                        )
                        nc.tensor.matmul(
                            op[:, ob * 512 : (ob + 1) * 512],
                            a_tiles[1][:, ss * 128 : (ss + 1) * 128],
                            w_o1[:, ob * 512 : (ob + 1) * 512],
                            start=False,
                            stop=True,
                        )
                    osb = opool.tile([128, D], BF16, tag="osb", name=f"osb{qb}_{ss}")
                    nc.vector.tensor_copy(osb[:], op[:])
                    nc.sync.dma_start(outp[r0 : r0 + 128, :], osb[:])
        if loop_ctx is not None:
            loop_ctx.__exit__(None, None, None)


def _build(loop_n=1):
    global _PROGRAM
    if loop_n != 1:
        nc = bacc.Bacc(
            "TRN2", target_bir_lowering=False, debug=False, num_devices=N_CORES
        )
        _emit(nc, loop_n)
        nc.compile()
        return nc
    if _PROGRAM is None:
        nc = bacc.Bacc(
            "TRN2", target_bir_lowering=False, debug=False, num_devices=N_CORES
        )
        _emit(nc)
        nc.compile()
        _PROGRAM = nc
    return _PROGRAM


def _rope_caches():
    j = np.arange(0, DK, 2, dtype=np.float32) / np.float32(DK)
    freqs = (1.0 / THETA**j).astype(np.float32)  # [32]
    t = np.arange(S, dtype=np.float32)
    ang = np.outer(t, freqs).astype(np.float32)  # [S, 32]
    return np.cos(ang), np.sin(ang)


def _make_masks():
    kk = np.arange(128)[:, None]
    jj = np.arange(128)[None, :]
    return (kk <= jj).astype(NPBF)


def _make_in_maps(x, token_positions, Wq, Wk, Wv, Wo):
    x = np.asarray(x, dtype=np.float32)
    token_positions = np.asarray(token_positions)
    Wq, Wk, Wv, Wo = (np.asarray(w, dtype=np.float32) for w in (Wq, Wk, Wv, Wo))

    cos_c, sin_c = _rope_caches()
    masks = _make_masks()

    in_maps = []
    for c in range(N_CORES):
        b, g = divmod(c, 4)
        heads = [g * HPC + hh for hh in range(HPC)]
        rows_e = np.concatenate([h * DK + np.arange(0, DK, 2) for h in heads])
        rows_o = rows_e + 1
        rows_v = np.concatenate([h * DK + np.arange(DK) for h in heads])

        pos = np.asarray(token_positions[b], dtype=np.int64)
        cosb = np.ascontiguousarray(cos_c[pos].T)  # [32, S]
        sinb = np.ascontiguousarray(sin_c[pos].T)

        in_maps.append(
            {
                "xT": np.ascontiguousarray(x[b].T).astype(NPBF),
                "wqeT": np.ascontiguousarray(Wq[rows_e].T).astype(NPBF),
                "wqoT": np.ascontiguousarray(Wq[rows_o].T).astype(NPBF),
                "wkeT": np.ascontiguousarray(Wk[rows_e].T).astype(NPBF),
                "wkoT": np.ascontiguousarray(Wk[rows_o].T).astype(NPBF),
                "wvT": np.ascontiguousarray(Wv[rows_v].T).astype(NPBF),
                "woT": np.ascontiguousarray(Wo[:, rows_v].T).astype(NPBF),
                "cos4": np.ascontiguousarray(np.tile(cosb, (4, 1))),
                "sin4": np.ascontiguousarray(np.tile(sinb, (4, 1))),
                "masks": masks,
            }
        )
    return in_maps


def kernel(x, token_positions, Wq, Wk, Wv, Wo):
    nc = _build()
    in_maps = _make_in_maps(x, token_positions, Wq, Wk, Wv, Wo)
    res = run_bass_kernel_spmd(nc, in_maps, list(range(N_CORES)))
    out = np.zeros((B, S, D), dtype=np.float32)
    for c in range(N_CORES):
        out[c // 4] += np.asarray(res.results[c]["out"], dtype=np.float32)
    return out


# revision 27
# speedup vs baseline: 1.1072x; 1.0196x over previous
"""Multi-head self-attention with RoPE on 8 Trainium2 NeuronCores.

Sharding: core c = batch(c // 4) x head-group(c % 4) -> 4 heads per core.
Each core computes attention for its 4 heads and a partial O-projection
(full [S, D] output restricted to its 256 input features); the host sums
the 4 partials per batch.

Device layout tricks:
  * x is transposed on host -> xT [D, S]; all projection matmuls contract
    over partitions without any on-device transpose.
  * Wq/Wk rows are permuted on host so even rope dims (E) and odd rope
    dims (O) of the 4 heads land in two separate 128-row projection
    outputs. RoPE then becomes lane-aligned elementwise DVE ops, and the
    Q.K contraction (invariant to the shared permutation) is done as two
    accumulating K=32 matmuls per head at distinct PE row-groups.
  * Scores are computed transposed (S_T[k, q]) so P_T feeds the PV matmul
    as the moving operand; a ones-column appended to V accumulates the
    softmax denominator in the same matmul. Softmax skips the max
    subtraction (scores are bounded ~|5|), exactly like exp-sum-divide.
  * All matmul operands are bf16 (fp32 PSUM accumulation): f32r moving
    operands stream at half rate and block fast-weight-load.
  * Projection (s-block sb) and attention (q-block sb) are interleaved in
    one loop so the PE always has independent matmul work -> HAM stays at
    K=8/8 (2.4 GHz) instead of oscillating cold.
  * exp runs on ACT for even heads and as a one-instruction Schraudolph
    bf16-bit-trick tensor_scalar on DVE for odd heads, halving exp wall
    time. Softmax normalizes with the same approximated values, so the
    denominator stays consistent.
"""

import os
import sys

sys.path.insert(0, "/opt/trn_rl_repo")

from contextlib import ExitStack

import ml_dtypes
import numpy as np

import concourse.bass as bass
import concourse.tile as tile
from concourse import bacc, mybir
from concourse.bass_utils import run_bass_kernel_spmd

B = 2
S = 4096
D = 1024
NH = 16
DK = 64
HPC = 4  # heads per core
N_CORES = 8
THETA = 10000.0
SBLK = 512  # s-block / q-block width
NSB = S // SBLK
KC = 128  # k chunk
F32 = mybir.dt.float32
BF16 = mybir.dt.bfloat16
I16 = mybir.dt.int16
NPBF = ml_dtypes.bfloat16
EXP = mybir.ActivationFunctionType.Exp
MULT = mybir.AluOpType.mult
ADD = mybir.AluOpType.add
# Schraudolph exp as bf16 bits: int16(x*0.125*128*log2e + 128*(127-C))
SCH_A = 128.0 * 1.4426950408889634 * 0.125
SCH_B = 128.0 * (127.0 - 0.04367)

_PROGRAM = None


def _emit(nc, loop_n=1):
    xT = nc.dram_tensor("xT", [D, S], BF16, kind="ExternalInput").ap()
    wqeT = nc.dram_tensor("wqeT", [D, 128], BF16, kind="ExternalInput").ap()
    wqoT = nc.dram_tensor("wqoT", [D, 128], BF16, kind="ExternalInput").ap()
    wkeT = nc.dram_tensor("wkeT", [D, 128], BF16, kind="ExternalInput").ap()
    wkoT = nc.dram_tensor("wkoT", [D, 128], BF16, kind="ExternalInput").ap()
    wvT = nc.dram_tensor("wvT", [D, 256], BF16, kind="ExternalInput").ap()
    woT = nc.dram_tensor("woT", [256, D], BF16, kind="ExternalInput").ap()
    cos4 = nc.dram_tensor("cos4", [128, S], F32, kind="ExternalInput").ap()
    sin4 = nc.dram_tensor("sin4", [128, S], F32, kind="ExternalInput").ap()
    masks = nc.dram_tensor("masks", [128, 128], BF16, kind="ExternalInput").ap()
    outp = nc.dram_tensor("out", [S, D], BF16, kind="ExternalOutput").ap()

    with tile.TileContext(nc) as tc, ExitStack() as ctx:
        wpool = ctx.enter_context(tc.tile_pool(name="w", bufs=1))
        xpool = ctx.enter_context(tc.tile_pool(name="x", bufs=24))
        qkpool = ctx.enter_context(tc.tile_pool(name="qk", bufs=1))
        vpool = ctx.enter_context(tc.tile_pool(name="v", bufs=1))
        trig = ctx.enter_context(tc.tile_pool(name="trig", bufs=4))
        tmp = ctx.enter_context(tc.tile_pool(name="tmp", bufs=4))
        ppool = ctx.enter_context(tc.tile_pool(name="p", bufs=4))
        apool = ctx.enter_context(tc.tile_pool(name="a", bufs=6))
        rpool = ctx.enter_context(tc.tile_pool(name="r", bufs=4))
        rbpool = ctx.enter_context(tc.tile_pool(name="rb", bufs=2))
        opool = ctx.enter_context(tc.tile_pool(name="ot", bufs=3))

        # ---- persistent SBUF tensors ----
        w_qe = wpool.tile([128, 8 * 128], BF16, tag="wqe")
        w_qo = wpool.tile([128, 8 * 128], BF16, tag="wqo")
        w_ke = wpool.tile([128, 8 * 128], BF16, tag="wke")
        w_ko = wpool.tile([128, 8 * 128], BF16, tag="wko")
        w_v = wpool.tile([128, 8 * 256], BF16, tag="wv")
        w_o0 = wpool.tile([128, D], BF16, tag="wo0")
        w_o1 = wpool.tile([128, D], BF16, tag="wo1")
        mask_t = wpool.tile([128, 128], BF16, tag="mask")

        for dst, src in (
            (w_qe, wqeT),
            (w_qo, wqoT),
            (w_ke, wkeT),
            (w_ko, wkoT),
            (w_v, wvT),
        ):
            nc.sync.dma_start(
                dst[:].rearrange("p (i f) -> p i f", i=8),
                src.rearrange("(i p) f -> p i f", p=128),
            )
        nc.sync.dma_start(w_o0[:], woT[0:128, :])
        nc.sync.dma_start(w_o1[:], woT[128:256, :])
        nc.sync.dma_start(mask_t[:], masks)

        qtE = qkpool.tile([128, S], BF16, tag="qtE")
        qtO = qkpool.tile([128, S], BF16, tag="qtO")
        ktE = qkpool.tile([128, S], BF16, tag="ktE")
        ktO = qkpool.tile([128, S], BF16, tag="ktO")
        # V with a ones column appended per head: [.. V_h (64) | 1 ..] x4
        vbig = vpool.tile([128, 32 * 260], BF16, tag="vbig")
        nc.vector.memset(vbig[:], 1.0)

        # ---- phase B: QKV projections + rope ----
        loop_ctx = tc.For_i(0, loop_n, 1) if loop_n > 1 else None
        if loop_ctx is not None:
            loop_ctx.__enter__()
        with (
            tc.tile_pool(name="projps", bufs=2, space="PSUM") as projps,
            tc.tile_pool(name="sps", bufs=2, space="PSUM") as sps,
            tc.tile_pool(name="accps", bufs=2, space="PSUM") as accps,
        ):
            for sb in range(NSB):
                scol = slice(sb * SBLK, (sb + 1) * SBLK)
                xt = []
                for i in range(8):
                    t = xpool.tile([128, SBLK], BF16, tag="xt")
                    nc.sync.dma_start(t[:], xT[i * 128 : (i + 1) * 128, scol])
                    xt.append(t)
                cos_t = trig.tile([128, SBLK], F32, tag="cos")
                sin_t = trig.tile([128, SBLK], F32, tag="sin")
                nc.sync.dma_start(cos_t[:], cos4[:, scol])
                nc.sync.dma_start(sin_t[:], sin4[:, scol])

                ps = {}
                for name, w in (("qe", w_qe), ("qo", w_qo), ("ke", w_ke), ("ko", w_ko)):
                    p = projps.tile([128, SBLK], F32, tag="proj")
                    for i in range(8):
                        nc.tensor.matmul(
                            p[:],
                            w[:, i * 128 : (i + 1) * 128],
                            xt[i][:],
                            start=(i == 0),
                            stop=(i == 7),
                        )
                    ps[name] = p

                # rope: E' = E cos - O sin ; O' = E sin + O cos
                for pe, po, dE, dO in (
                    (ps["qe"], ps["qo"], qtE, qtO),
                    (ps["ke"], ps["ko"], ktE, ktO),
                ):
                    t1 = tmp.tile([128, SBLK], F32, tag="t1")
                    t2 = tmp.tile([128, SBLK], F32, tag="t2")
                    nc.vector.tensor_mul(t1[:], pe[:], cos_t[:])
                    nc.vector.tensor_mul(t2[:], po[:], sin_t[:])
                    nc.vector.tensor_sub(dE[:, scol], t1[:], t2[:])
                    t3 = tmp.tile([128, SBLK], F32, tag="t1")
                    t4 = tmp.tile([128, SBLK], F32, tag="t2")
                    nc.vector.tensor_mul(t3[:], pe[:], sin_t[:])
                    nc.vector.tensor_mul(t4[:], po[:], cos_t[:])
                    nc.vector.tensor_add(dO[:, scol], t3[:], t4[:])

                # V for the 4 k-chunks of this s-block
                for ss in range(4):
                    vp = projps.tile([128, SBLK], F32, tag="proj")
                    for i in range(8):
                        nc.tensor.matmul(
                            vp[:, 0:256],
                            xt[i][:, ss * 128 : (ss + 1) * 128],
                            w_v[:, i * 256 : (i + 1) * 256],
                            start=(i == 0),
                            stop=(i == 7),
                        )
                    kc = sb * 4 + ss
                    nc.vector.tensor_copy(
                        vbig[:, kc * 260 : (kc + 1) * 260].rearrange(
                            "p (h f) -> p h f", h=4
                        )[:, :, 0:64],
                        vp[:, 0:256].rearrange("p (h f) -> p h f", h=4),
                    )

                # ---- attention for q-block qb = sb ----
                qb = sb
                a_tiles = [
                    apool.tile([128, SBLK], BF16, tag="a", name=f"a{qb}_{i}")
                    for i in range(2)
                ]
                nk = 4 * qb + 4
                for hp in range(2):
                    heads = (2 * hp, 2 * hp + 1)
                    accs = [
                        accps.tile([65, SBLK], F32, tag="acc", name=f"acc{qb}_{h}")
                        for h in heads
                    ]
                    for cc in range(nk // 2):
                        chunks = (2 * cc, 2 * cc + 1)
                        # d = masked-out leading q-columns of each chunk
                        ds = [max(0, (c - 4 * qb)) * KC for c in chunks]
                        sp_pair = [
                            sps.tile(
                                [128, 2 * SBLK], F32, tag="s", name=f"s{qb}_{cc}_{h}"
                            )
                            for h in heads
                        ]
                        # head-inner order: consecutive matmuls hit different
                        # PE row-groups so they execute concurrently
                        for kt, qt, st0, st1 in (
                            (ktE, qtE, True, False),
                            (ktO, qtO, False, True),
                        ):
                            for ci, c in enumerate(chunks):
                                d = ds[ci]
                                for hi, h in enumerate(heads):
                                    hr = slice(h * 32, (h + 1) * 32)
                                    nc.tensor.matmul(
                                        sp_pair[hi][:, ci * SBLK + d : (ci + 1) * SBLK],
                                        kt[hr, c * KC : (c + 1) * KC],
                                        qt[hr, qb * SBLK + d : (qb + 1) * SBLK],
                                        start=st0,
                                        stop=st1,
                                        tile_position=(h * 32, 0),
                                    )
                        pts = []
                        for hi, h in enumerate(heads):
                            pt = ppool.tile([128, 2 * SBLK], BF16, tag="pt")
                            if hi == 0 or cc % 2 == 1:
                                nc.scalar.activation(
                                    pt[:, ds[0] :],
                                    sp_pair[hi][:, ds[0] :],
                                    EXP,
                                    scale=0.125,
                                )
                            else:
                                nc.vector.tensor_scalar(
                                    pt[:, ds[0] :].bitcast(I16),
                                    sp_pair[hi][:, ds[0] :],
                                    SCH_A,
                                    SCH_B,
                                    op0=MULT,
                                    op1=ADD,
                                )
                            for ci, c in enumerate(chunks):
                                d = ds[ci]
                                if c >= 4 * qb:
                                    # diagonal 128x128 triangle of this chunk
                                    nc.vector.tensor_mul(
                                        pt[:, ci * SBLK + d : ci * SBLK + d + KC],
                                        pt[:, ci * SBLK + d : ci * SBLK + d + KC],
                                        mask_t[:],
                                    )
                            pts.append(pt)
                        for hi, h in enumerate(heads):
                            pt = pts[hi]
                            for ci, c in enumerate(chunks):
                                d = ds[ci]
                                nc.tensor.matmul(
                                    accs[hi][0:65, d:SBLK],
                                    vbig[:, c * 260 + h * 65 : c * 260 + (h + 1) * 65],
                                    pt[:, ci * SBLK + d : (ci + 1) * SBLK],
                                    start=(c == 0),
                                    stop=(c == nk - 1),
                                )
                    for hi, h in enumerate(heads):
                        den = rpool.tile([1, SBLK], F32, tag="den")
                        nc.vector.tensor_copy(den[:], accs[hi][64:65, :])
                        recip = rpool.tile([1, SBLK], F32, tag="recip")
                        nc.vector.reciprocal_approx_fast(recip[:], den[:])
                        rb = rbpool.tile([64, SBLK], F32, tag="rb")
                        nc.gpsimd.partition_broadcast(rb[:], recip[0:1, :])
                        nc.vector.tensor_mul(
                            a_tiles[h // 2][(h % 2) * 64 : (h % 2 + 1) * 64, :],
                            accs[hi][0:64, :],
                            rb[:],
                        )
                # O projection for this q-block (borrows sps psum slots)
                for ss in range(4):
                    r0 = qb * SBLK + ss * 128
                    op = sps.tile([128, 2 * SBLK], F32, tag="s", name=f"o{qb}_{ss}")
                    for ob in range(2):
                        nc.tensor.matmul(
                            op[:, ob * 512 : (ob + 1) * 512],
                            a_tiles[0][:, ss * 128 : (ss + 1) * 128],
                            w_o0[:, ob * 512 : (ob + 1) * 512],
                            start=True,
                            stop=False,
                        )
                        nc.tensor.matmul(
                            op[:, ob * 512 : (ob + 1) * 512],
                            a_tiles[1][:, ss * 128 : (ss + 1) * 128],
                            w_o1[:, ob * 512 : (ob + 1) * 512],
                            start=False,
                            stop=True,
                        )
                    osb = opool.tile([128, D], BF16, tag="osb", name=f"osb{qb}_{ss}")
                    nc.vector.tensor_copy(osb[:], op[:])
                    nc.sync.dma_start(outp[r0 : r0 + 128, :], osb[:])
        if loop_ctx is not None:
            loop_ctx.__exit__(None, None, None)


def _build(loop_n=1):
    global _PROGRAM
    if loop_n != 1:
        nc = bacc.Bacc(
            "TRN2", target_bir_lowering=False, debug=False, num_devices=N_CORES
        )
        _emit(nc, loop_n)
        nc.compile()
        return nc
    if _PROGRAM is None:
        nc = bacc.Bacc(
            "TRN2", target_bir_lowering=False, debug=False, num_devices=N_CORES
        )
        _emit(nc)
        nc.compile()
        _PROGRAM = nc
    return _PROGRAM


def _rope_caches():
    j = np.arange(0, DK, 2, dtype=np.float32) / np.float32(DK)
    freqs = (1.0 / THETA**j).astype(np.float32)  # [32]
    t = np.arange(S, dtype=np.float32)
    ang = np.outer(t, freqs).astype(np.float32)  # [S, 32]
    return np.cos(ang), np.sin(ang)


def _make_masks():
    kk = np.arange(128)[:, None]
    jj = np.arange(128)[None, :]
    return (kk <= jj).astype(NPBF)


def _make_in_maps(x, token_positions, Wq, Wk, Wv, Wo):
    x = np.asarray(x, dtype=np.float32)
    token_positions = np.asarray(token_positions)
    Wq, Wk, Wv, Wo = (np.asarray(w, dtype=np.float32) for w in (Wq, Wk, Wv, Wo))

    cos_c, sin_c = _rope_caches()
    masks = _make_masks()

    in_maps = []
    for c in range(N_CORES):
        b, g = divmod(c, 4)
        heads = [g * HPC + hh for hh in range(HPC)]
        rows_e = np.concatenate([h * DK + np.arange(0, DK, 2) for h in heads])
        rows_o = rows_e + 1
        rows_v = np.concatenate([h * DK + np.arange(DK) for h in heads])

        pos = np.asarray(token_positions[b], dtype=np.int64)
        cosb = np.ascontiguousarray(cos_c[pos].T)  # [32, S]
        sinb = np.ascontiguousarray(sin_c[pos].T)

        in_maps.append(
            {
                "xT": np.ascontiguousarray(x[b].T).astype(NPBF),
                "wqeT": np.ascontiguousarray(Wq[rows_e].T).astype(NPBF),
                "wqoT": np.ascontiguousarray(Wq[rows_o].T).astype(NPBF),
                "wkeT": np.ascontiguousarray(Wk[rows_e].T).astype(NPBF),
                "wkoT": np.ascontiguousarray(Wk[rows_o].T).astype(NPBF),
                "wvT": np.ascontiguousarray(Wv[rows_v].T).astype(NPBF),
                "woT": np.ascontiguousarray(Wo[:, rows_v].T).astype(NPBF),
                "cos4": np.ascontiguousarray(np.tile(cosb, (4, 1))),
                "sin4": np.ascontiguousarray(np.tile(sinb, (4, 1))),
                "masks": masks,
            }
        )
    return in_maps


def kernel(x, token_positions, Wq, Wk, Wv, Wo):
    nc = _build()
    in_maps = _make_in_maps(x, token_positions, Wq, Wk, Wv, Wo)
    res = run_bass_kernel_spmd(nc, in_maps, list(range(N_CORES)))
    out = np.zeros((B, S, D), dtype=np.float32)
    for c in range(N_CORES):
        out[c // 4] += np.asarray(res.results[c]["out"], dtype=np.float32)
    return out
